# revision 6
# baseline (speedup 1.0000x reference)
"""YOLO anchor-box decode (predictTransform) as a Trainium2 Bass/Tile kernel.

Input : prediction [32, 255, 76, 76] f32, anchors [3,2] f32, inputDim, numClasses
Output: [32, 17328, 85] f32   (decoded boxes in input-image pixel units)

Math per batch (stride = inputDim // 76 = 8, attrs = 85, A = 3 anchors):
  view [255, 5776] -> transpose -> [5776, 255] rows g = (gy*76+gx), cols (a, k)
  k=0: (sigmoid(x) + gx) * stride      k=1: (sigmoid(y) + gy) * stride
  k=2: exp(w) * anchors[a,0]           k=3: exp(h) * anchors[a,1]
  k>=4: sigmoid(.)

Distribution: pure data parallel over batch, 4 batches per core on 8 cores.

Per-core dataflow (memory-bound problem, ~47 MB traffic/core):
  1. DMA each batch's [255, 5776] into SBUF as [128, 5776] + [127, 5776].
  2. TensorE transposes 128x128 fp32 blocks (identity matmul) into PSUM,
     8 g-blocks per PSUM group tile (256-col stride keeps each matmul
     output inside one PSUM bank).
  3. ScalarE reads PSUM and writes SBUF output tiles applying tanh(x/2)
     (whole tile) and exp (w/h cols -> small staging tile).  Using tanh
     instead of sigmoid keeps every ACT op inside the single
     `exp_and_others` table set: sigmoid and exp live in different ACT
     table sets and each switch costs ~2.7us.
  4. VectorE: sigmoid = 0.5*tanh + 0.5 (one fused mul-add pass), then the
     x/y affine (x stride, + stride*grid offset from a precomputed table)
     and w/h anchor multiply.
  5. DMA out [128 g-rows, nb*255] tiles; HBM side is contiguous per row.
"""

import os

import numpy as np

import concourse.bacc as bacc
import concourse.bass_utils as bass_utils
import concourse.mybir as mybir
import concourse.tile as tile

F32 = mybir.dt.float32

B, CH, G, G2, A, ATT = 32, 255, 76, 5776, 3, 85
NCORES, BPC = 8, 4            # cores, batches per core
TAIL = G2 - 45 * 128          # 16 leftover grid cells per batch
GROUPS = [(0, 8), (8, 8), (16, 8), (24, 8), (32, 8), (40, 6)]  # (first block, nblocks)
PSTRIDE = 256                 # per-block PSUM column stride (bank-safe for 255 cols)

_PROGRAMS = {}
LAST_RESULTS = None


def _build_program(stride: float):
    nc = bacc.Bacc(
        "TRN2",
        target_bir_lowering=False,
        debug=False,
        enable_asserts=False,
        num_devices=NCORES,
    )
    pred = nc.dram_tensor("pred", [BPC, CH, G2], F32, kind="ExternalInput").ap()
    gxy = nc.dram_tensor("gxy", [128, 46 * 6], F32, kind="ExternalInput").ap()
    ancf = nc.dram_tensor("ancf", [128, 48], F32, kind="ExternalInput").ap()
    ident = nc.dram_tensor("ident", [128, 128], F32, kind="ExternalInput").ap()
    out = nc.dram_tensor("out", [BPC, G2 * A, ATT], F32, kind="ExternalOutput").ap()
    out_flat = out.rearrange("b r k -> b (r k)")

    with tile.TileContext(nc) as tc:
        with (
            tc.tile_pool(name="consts", bufs=1) as consts,
            tc.tile_pool(name="inpool", bufs=2) as inpool,
            tc.tile_pool(name="outpool", bufs=3) as outpool,
            tc.tile_pool(name="whpool", bufs=3) as whpool,
            tc.tile_pool(name="pspool", bufs=2, space="PSUM") as pspool,
        ):
            ident_t = consts.tile([128, 128], F32)
            nc.sync.dma_start(out=ident_t, in_=ident)
            gxy_t = consts.tile([128, 46 * 6], F32)
            nc.sync.dma_start(out=gxy_t, in_=gxy)
            ancf_t = consts.tile([128, 48], F32)
            nc.sync.dma_start(out=ancf_t, in_=ancf)

            for b in range(BPC):
                in0 = inpool.tile([128, G2], F32, tag="in0")
                in1 = inpool.tile([127, G2], F32, tag="in1")
                # One 11.5 KB descriptor per partition skews badly across the
                # 16 SDMA engines (measured: one engine carried 53% of every
                # such load -> 27 GB/s).  Interleaved 361-elem (1444 B)
                # chunks force a non-mergeable 3-dim AP with several
                # descriptors per partition, which balances like the stores
                # do.  h splits halves (pipelining granularity), t is the
                # interleave parity that defeats the contiguity merge.
                for src_rows, dst in ((pred[b, 0:128, :], in0), (pred[b, 128:CH, :], in1)):
                    sv = src_rows.rearrange("p (h s t q) -> p h t s q", h=2, t=2, q=361)
                    dv = dst.rearrange("p (h s t q) -> p h t s q", h=2, t=2, q=361)
                    for h in range(2):
                        for t in range(2):
                            nc.sync.dma_start(out=dv[:, h, t], in_=sv[:, h, t])

                for j0, nb in GROUPS:
                    ps = pspool.tile([128, 8 * PSTRIDE], F32, tag="ps")
                    outt = outpool.tile([128, 8 * 255], F32, tag="outt")
                    wht = whpool.tile([128, 8 * 6], F32, tag="wht")
                    for jj in range(nb):
                        g0 = (j0 + jj) * 128
                        gcnt = min(128, G2 - g0)
                        po = jj * PSTRIDE
                        if gcnt < 128:
                            # Tail block: the ACT/DVE ops below read all 128
                            # partitions of this column range; zero it first
                            # (engine ops can't start at partition 16) so
                            # nothing reads stale PSUM, then let the
                            # transposes overwrite rows 0..gcnt.  Rows >=
                            # gcnt are never stored to DRAM.
                            nc.vector.memset(ps[:, po : po + 255], 0.0)
                        nc.tensor.transpose(
                            ps[0:gcnt, po : po + 128],
                            in0[:, g0 : g0 + gcnt],
                            ident_t,
                        )
                        nc.tensor.transpose(
                            ps[0:gcnt, po + 128 : po + 255],
                            in1[:, g0 : g0 + gcnt],
                            ident_t[0:127, 0:127],
                        )
                    nw = nb * 255
                    ps_v = ps[:, 0 : nb * PSTRIDE].rearrange(
                        "p (j c) -> p j c", c=PSTRIDE
                    )[:, :, 0:255]
                    out_v = outt[:, 0:nw].rearrange("p (j c) -> p j c", c=255)
                    nc.scalar.activation(
                        out_v, ps_v, mybir.ActivationFunctionType.Tanh, scale=0.5
                    )
                    ps_wh = ps_v.rearrange("p j (a k) -> p j a k", a=A)[:, :, :, 2:4]
                    wh_v = wht[:, 0 : nb * 6].rearrange(
                        "p (j a k) -> p j a k", a=A, k=2
                    )
                    nc.scalar.activation(
                        wh_v, ps_wh, mybir.ActivationFunctionType.Exp
                    )
                    # sigmoid = 0.5*tanh + 0.5, fused single pass
                    nc.vector.tensor_scalar(
                        out=outt[:, 0:nw],
                        in0=outt[:, 0:nw],
                        scalar1=0.5,
                        scalar2=0.5,
                        op0=mybir.AluOpType.mult,
                        op1=mybir.AluOpType.add,
                    )
                    out4 = out_v.rearrange("p j (a k) -> p j a k", a=A)
                    xy = out4[:, :, :, 0:2]
                    nc.vector.tensor_scalar_mul(xy, xy, float(stride))
                    gxy_v = gxy_t[:, j0 * 6 : (j0 + nb) * 6].rearrange(
                        "p (j a k) -> p j a k", a=A, k=2
                    )
                    nc.vector.tensor_add(xy, xy, gxy_v)
                    whc = out4[:, :, :, 2:4]
                    anc_v = ancf_t[:, 0 : nb * 6].rearrange(
                        "p (j a k) -> p j a k", a=A, k=2
                    )
                    nc.vector.tensor_mul(whc, wh_v, anc_v)

                    nfull = nb if (j0 + nb) * 128 <= G2 else nb - 1
                    base = j0 * 128 * 255
                    dst = out_flat[b, base : base + nfull * 128 * 255].rearrange(
                        "(j p c) -> p j c", p=128, c=255
                    )
                    src = outt[:, 0 : nfull * 255].rearrange("p (j c) -> p j c", c=255)
                    nc.scalar.dma_start(out=dst, in_=src)
                    if nfull != nb:
                        tb = base + nfull * 128 * 255
                        dst_t = out_flat[b, tb : tb + TAIL * 255].rearrange(
                            "(p c) -> p c", c=255
                        )
                        nc.scalar.dma_start(
                            out=dst_t,
                            in_=outt[0:TAIL, nfull * 255 : (nfull + 1) * 255],
                        )
    nc.compile()
    return nc


def _tables(stride: float, anchors: np.ndarray):
    g = np.arange(46 * 128, dtype=np.int64)
    gx = (g % G).astype(np.float32) * stride
    gy = (g // G).astype(np.float32) * stride
    gx[g >= G2] = 0.0
    gy[g >= G2] = 0.0
    gxy = np.stack([gx.reshape(46, 128).T, gy.reshape(46, 128).T], axis=-1)
    gxy = np.repeat(gxy[:, :, None, :], A, axis=2)  # [128, 46, 3, 2]
    gxy = np.ascontiguousarray(gxy.reshape(128, 46 * 6), dtype=np.float32)
    ancf = np.ascontiguousarray(
        np.broadcast_to(
            anchors.astype(np.float32)[None, None], (128, 8, A, 2)
        ).reshape(128, 48)
    )
    ident = np.eye(128, dtype=np.float32)
    return gxy, ancf, ident


def get_program(stride: float):
    key = float(stride)
    if key not in _PROGRAMS:
        _PROGRAMS[key] = _build_program(key)
    return _PROGRAMS[key]


def core_inputs(prediction, anchors, inputDim):
    """Host-side prep: per-core input dicts (exposed for testing)."""
    pred = np.asarray(prediction, dtype=np.float32)
    anc = np.asarray(anchors, dtype=np.float32)
    input_dim = int(np.asarray(inputDim))
    assert pred.shape == (B, CH, G, G), pred.shape
    assert anc.shape == (A, 2), anc.shape
    stride = input_dim // G
    predf = pred.reshape(B, CH, G2)
    gxy, ancf, ident = _tables(float(stride), anc)
    in_maps = [
        {
            "pred": np.ascontiguousarray(predf[i * BPC : (i + 1) * BPC]),
            "gxy": gxy,
            "ancf": ancf,
            "ident": ident,
        }
        for i in range(NCORES)
    ]
    return in_maps, stride


def kernel(prediction, anchors, inputDim, numClasses):
    global LAST_RESULTS
    assert int(np.asarray(numClasses)) == ATT - 5
    in_maps, stride = core_inputs(prediction, anchors, inputDim)
    nc = get_program(float(stride))
    kwargs = {}
    if int(os.environ.get("KERNEL_TRACE", "0")):
        kwargs = dict(trace=True, trace_cores=[0])
    res = bass_utils.run_bass_kernel_spmd(
        nc, in_maps, core_ids=list(range(NCORES)), **kwargs
    )
    LAST_RESULTS = res
    return np.concatenate([r["out"] for r in res.results], axis=0)


# revision 7
# speedup vs baseline: 1.9167x; 1.9167x over previous
"""YOLO anchor-box decode (predictTransform) as a Trainium2 Bass/Tile kernel.

Input : prediction [32, 255, 76, 76] f32, anchors [3,2] f32, inputDim, numClasses
Output: [32, 17328, 85] f32   (decoded boxes in input-image pixel units)

Math per batch (stride = inputDim // 76 = 8, attrs = 85, A = 3 anchors):
  view [255, 5776] -> transpose -> [5776, 255] rows g = (gy*76+gx), cols (a, k)
  k=0: (sigmoid(x) + gx) * stride      k=1: (sigmoid(y) + gy) * stride
  k=2: exp(w) * anchors[a,0]           k=3: exp(h) * anchors[a,1]
  k>=4: sigmoid(.)

Distribution: pure data parallel over batch, 4 batches per core on 8 cores.

Per-core dataflow (memory-bound problem, ~47 MB traffic/core):
  1. DMA each batch's [255, 5776] into SBUF as [128, 5776] + [127, 5776].
  2. TensorE transposes 128x128 fp32 blocks (identity matmul) into PSUM,
     8 g-blocks per PSUM group tile (256-col stride keeps each matmul
     output inside one PSUM bank).
  3. ScalarE reads PSUM and writes SBUF output tiles applying tanh(x/2)
     (whole tile) and exp (w/h cols -> small staging tile).  Using tanh
     instead of sigmoid keeps every ACT op inside the single
     `exp_and_others` table set: sigmoid and exp live in different ACT
     table sets and each switch costs ~2.7us.
  4. VectorE: sigmoid = 0.5*tanh + 0.5 (one fused mul-add pass), then the
     x/y affine (x stride, + stride*grid offset from a precomputed table)
     and w/h anchor multiply.
  5. DMA out [128 g-rows, nb*255] tiles; HBM side is contiguous per row.
"""

import os

import numpy as np

import concourse.bacc as bacc
import concourse.bass_utils as bass_utils
import concourse.mybir as mybir
import concourse.tile as tile

F32 = mybir.dt.float32

B, CH, G, G2, A, ATT = 32, 255, 76, 5776, 3, 85
NCORES, BPC = 8, 4            # cores, batches per core
TAIL = G2 - 45 * 128          # 16 leftover grid cells per batch
GROUPS = [(0, 8), (8, 8), (16, 8), (24, 8), (32, 8), (40, 6)]  # (first block, nblocks)
PSTRIDE = 256                 # per-block PSUM column stride (bank-safe for 255 cols)

_PROGRAMS = {}
LAST_RESULTS = None


def _build_program(stride: float):
    nc = bacc.Bacc(
        "TRN2",
        target_bir_lowering=False,
        debug=False,
        enable_asserts=False,
        num_devices=NCORES,
    )
    pred = nc.dram_tensor("pred", [BPC, CH, G2], F32, kind="ExternalInput").ap()
    gxy = nc.dram_tensor("gxy", [128, 46 * 6], F32, kind="ExternalInput").ap()
    ancf = nc.dram_tensor("ancf", [128, 48], F32, kind="ExternalInput").ap()
    ident = nc.dram_tensor("ident", [128, 128], F32, kind="ExternalInput").ap()
    out = nc.dram_tensor("out", [BPC, G2 * A, ATT], F32, kind="ExternalOutput").ap()
    out_flat = out.rearrange("b r k -> b (r k)")

    with tile.TileContext(nc) as tc:
        with (
            tc.tile_pool(name="consts", bufs=1) as consts,
            tc.tile_pool(name="inpool", bufs=2) as inpool,
            tc.tile_pool(name="outpool", bufs=3) as outpool,
            tc.tile_pool(name="whpool", bufs=3) as whpool,
            tc.tile_pool(name="pspool", bufs=2, space="PSUM") as pspool,
        ):
            ident_t = consts.tile([128, 128], F32)
            nc.sync.dma_start(out=ident_t, in_=ident)
            gxy_t = consts.tile([128, 46 * 6], F32)
            nc.sync.dma_start(out=gxy_t, in_=gxy)
            ancf_t = consts.tile([128, 48], F32)
            nc.sync.dma_start(out=ancf_t, in_=ancf)

            for b in range(BPC):
                in0 = inpool.tile([128, G2], F32, tag="in0")
                in1 = inpool.tile([127, G2], F32, tag="in1")
                # Loads go via SWDGE (gpsimd): HWDGE HBM->SBUF loads
                # round-robin descriptors over 32 slots of which the upper 16
                # collapse onto one SDMA engine (measured: 17/32 = 53% of
                # load bytes on engine 64 at 27 GB/s, at any descriptor
                # size).  SWDGE's partition swizzle balances all 16 engines.
                for lo, hi in ((0, G2 // 2), (G2 // 2, G2)):
                    nc.gpsimd.dma_start(out=in0[:, lo:hi], in_=pred[b, 0:128, lo:hi])
                    nc.gpsimd.dma_start(out=in1[:, lo:hi], in_=pred[b, 128:CH, lo:hi])

                for j0, nb in GROUPS:
                    ps = pspool.tile([128, 8 * PSTRIDE], F32, tag="ps")
                    outt = outpool.tile([128, 8 * 255], F32, tag="outt")
                    wht = whpool.tile([128, 8 * 6], F32, tag="wht")
                    for jj in range(nb):
                        g0 = (j0 + jj) * 128
                        gcnt = min(128, G2 - g0)
                        po = jj * PSTRIDE
                        if gcnt < 128:
                            # Tail block: the ACT/DVE ops below read all 128
                            # partitions of this column range; zero it first
                            # (engine ops can't start at partition 16) so
                            # nothing reads stale PSUM, then let the
                            # transposes overwrite rows 0..gcnt.  Rows >=
                            # gcnt are never stored to DRAM.
                            nc.vector.memset(ps[:, po : po + 255], 0.0)
                        nc.tensor.transpose(
                            ps[0:gcnt, po : po + 128],
                            in0[:, g0 : g0 + gcnt],
                            ident_t,
                        )
                        nc.tensor.transpose(
                            ps[0:gcnt, po + 128 : po + 255],
                            in1[:, g0 : g0 + gcnt],
                            ident_t[0:127, 0:127],
                        )
                    nw = nb * 255
                    ps_v = ps[:, 0 : nb * PSTRIDE].rearrange(
                        "p (j c) -> p j c", c=PSTRIDE
                    )[:, :, 0:255]
                    out_v = outt[:, 0:nw].rearrange("p (j c) -> p j c", c=255)
                    nc.scalar.activation(
                        out_v, ps_v, mybir.ActivationFunctionType.Tanh, scale=0.5
                    )
                    ps_wh = ps_v.rearrange("p j (a k) -> p j a k", a=A)[:, :, :, 2:4]
                    wh_v = wht[:, 0 : nb * 6].rearrange(
                        "p (j a k) -> p j a k", a=A, k=2
                    )
                    nc.scalar.activation(
                        wh_v, ps_wh, mybir.ActivationFunctionType.Exp
                    )
                    # sigmoid = 0.5*tanh + 0.5, fused single pass
                    nc.vector.tensor_scalar(
                        out=outt[:, 0:nw],
                        in0=outt[:, 0:nw],
                        scalar1=0.5,
                        scalar2=0.5,
                        op0=mybir.AluOpType.mult,
                        op1=mybir.AluOpType.add,
                    )
                    out4 = out_v.rearrange("p j (a k) -> p j a k", a=A)
                    xy = out4[:, :, :, 0:2]
                    nc.vector.tensor_scalar_mul(xy, xy, float(stride))
                    gxy_v = gxy_t[:, j0 * 6 : (j0 + nb) * 6].rearrange(
                        "p (j a k) -> p j a k", a=A, k=2
                    )
                    nc.vector.tensor_add(xy, xy, gxy_v)
                    whc = out4[:, :, :, 2:4]
                    anc_v = ancf_t[:, 0 : nb * 6].rearrange(
                        "p (j a k) -> p j a k", a=A, k=2
                    )
                    nc.vector.tensor_mul(whc, wh_v, anc_v)

                    nfull = nb if (j0 + nb) * 128 <= G2 else nb - 1
                    base = j0 * 128 * 255
                    dst = out_flat[b, base : base + nfull * 128 * 255].rearrange(
                        "(j p c) -> p j c", p=128, c=255
                    )
                    src = outt[:, 0 : nfull * 255].rearrange("p (j c) -> p j c", c=255)
                    nc.scalar.dma_start(out=dst, in_=src)
                    if nfull != nb:
                        tb = base + nfull * 128 * 255
                        dst_t = out_flat[b, tb : tb + TAIL * 255].rearrange(
                            "(p c) -> p c", c=255
                        )
                        nc.scalar.dma_start(
                            out=dst_t,
                            in_=outt[0:TAIL, nfull * 255 : (nfull + 1) * 255],
                        )
    nc.compile()
    return nc


def _tables(stride: float, anchors: np.ndarray):
    g = np.arange(46 * 128, dtype=np.int64)
    gx = (g % G).astype(np.float32) * stride
    gy = (g // G).astype(np.float32) * stride
    gx[g >= G2] = 0.0
    gy[g >= G2] = 0.0
    gxy = np.stack([gx.reshape(46, 128).T, gy.reshape(46, 128).T], axis=-1)
    gxy = np.repeat(gxy[:, :, None, :], A, axis=2)  # [128, 46, 3, 2]
    gxy = np.ascontiguousarray(gxy.reshape(128, 46 * 6), dtype=np.float32)
    ancf = np.ascontiguousarray(
        np.broadcast_to(
            anchors.astype(np.float32)[None, None], (128, 8, A, 2)
        ).reshape(128, 48)
    )
    ident = np.eye(128, dtype=np.float32)
    return gxy, ancf, ident


def get_program(stride: float):
    key = float(stride)
    if key not in _PROGRAMS:
        _PROGRAMS[key] = _build_program(key)
    return _PROGRAMS[key]


def core_inputs(prediction, anchors, inputDim):
    """Host-side prep: per-core input dicts (exposed for testing)."""
    pred = np.asarray(prediction, dtype=np.float32)
    anc = np.asarray(anchors, dtype=np.float32)
    input_dim = int(np.asarray(inputDim))
    assert pred.shape == (B, CH, G, G), pred.shape
    assert anc.shape == (A, 2), anc.shape
    stride = input_dim // G
    predf = pred.reshape(B, CH, G2)
    gxy, ancf, ident = _tables(float(stride), anc)
    in_maps = [
        {
            "pred": np.ascontiguousarray(predf[i * BPC : (i + 1) * BPC]),
            "gxy": gxy,
            "ancf": ancf,
            "ident": ident,
        }
        for i in range(NCORES)
    ]
    return in_maps, stride


def kernel(prediction, anchors, inputDim, numClasses):
    global LAST_RESULTS
    assert int(np.asarray(numClasses)) == ATT - 5
    in_maps, stride = core_inputs(prediction, anchors, inputDim)
    nc = get_program(float(stride))
    kwargs = {}
    if int(os.environ.get("KERNEL_TRACE", "0")):
        kwargs = dict(trace=True, trace_cores=[0])
    res = bass_utils.run_bass_kernel_spmd(
        nc, in_maps, core_ids=list(range(NCORES)), **kwargs
    )
    LAST_RESULTS = res
    return np.concatenate([r["out"] for r in res.results], axis=0)


# revision 8
# speedup vs baseline: 2.0256x; 1.0568x over previous
"""YOLO anchor-box decode (predictTransform) as a Trainium2 Bass/Tile kernel.

Input : prediction [32, 255, 76, 76] f32, anchors [3,2] f32, inputDim, numClasses
Output: [32, 17328, 85] f32   (decoded boxes in input-image pixel units)

Math per batch (stride = inputDim // 76 = 8, attrs = 85, A = 3 anchors):
  view [255, 5776] -> transpose -> [5776, 255] rows g = (gy*76+gx), cols (a, k)
  k=0: (sigmoid(x) + gx) * stride      k=1: (sigmoid(y) + gy) * stride
  k=2: exp(w) * anchors[a,0]           k=3: exp(h) * anchors[a,1]
  k>=4: sigmoid(.)

Distribution: pure data parallel over batch, 4 batches per core on 8 cores.

Per-core dataflow (memory-bound problem, ~47 MB traffic/core):
  1. DMA each batch's [255, 5776] into SBUF as [128, 5776] + [127, 5776].
  2. TensorE transposes 128x128 fp32 blocks (identity matmul) into PSUM,
     8 g-blocks per PSUM group tile (256-col stride keeps each matmul
     output inside one PSUM bank).
  3. ScalarE reads PSUM and writes SBUF output tiles applying tanh(x/2)
     (whole tile) and exp (w/h cols -> small staging tile).  Using tanh
     instead of sigmoid keeps every ACT op inside the single
     `exp_and_others` table set: sigmoid and exp live in different ACT
     table sets and each switch costs ~2.7us.
  4. VectorE: sigmoid = 0.5*tanh + 0.5 (one fused mul-add pass), then the
     x/y affine (x stride, + stride*grid offset from a precomputed table)
     and w/h anchor multiply.
  5. DMA out [128 g-rows, nb*255] tiles; HBM side is contiguous per row.
"""

import os

import numpy as np

import concourse.bacc as bacc
import concourse.bass_utils as bass_utils
import concourse.mybir as mybir
import concourse.tile as tile

F32 = mybir.dt.float32

B, CH, G, G2, A, ATT = 32, 255, 76, 5776, 3, 85
NCORES, BPC = 8, 4            # cores, batches per core
TAIL = G2 - 45 * 128          # 16 leftover grid cells per batch
GROUPS = [(0, 8), (8, 8), (16, 8), (24, 8), (32, 8), (40, 6)]  # (first block, nblocks)
PSTRIDE = 256                 # per-block PSUM column stride (bank-safe for 255 cols)

_PROGRAMS = {}
LAST_RESULTS = None


def _build_program(stride: float):
    nc = bacc.Bacc(
        "TRN2",
        target_bir_lowering=False,
        debug=False,
        enable_asserts=False,
        num_devices=NCORES,
    )
    pred = nc.dram_tensor("pred", [BPC, CH, G2], F32, kind="ExternalInput").ap()
    gxy = nc.dram_tensor("gxy", [128, 46 * 6], F32, kind="ExternalInput").ap()
    ancf = nc.dram_tensor("ancf", [128, 48], F32, kind="ExternalInput").ap()
    ident = nc.dram_tensor("ident", [128, 128], F32, kind="ExternalInput").ap()
    out = nc.dram_tensor("out", [BPC, G2 * A, ATT], F32, kind="ExternalOutput").ap()
    out_flat = out.rearrange("b r k -> b (r k)")

    with tile.TileContext(nc) as tc:
        with (
            tc.tile_pool(name="consts", bufs=1) as consts,
            tc.tile_pool(name="inpool", bufs=2) as inpool,
            tc.tile_pool(name="outpool", bufs=3) as outpool,
            tc.tile_pool(name="whpool", bufs=3) as whpool,
            tc.tile_pool(name="pspool", bufs=2, space="PSUM") as pspool,
        ):
            ident_t = consts.tile([128, 128], F32)
            nc.sync.dma_start(out=ident_t, in_=ident)
            gxy_t = consts.tile([128, 46 * 6], F32)
            nc.sync.dma_start(out=gxy_t, in_=gxy)
            ancf_t = consts.tile([128, 48], F32)
            nc.sync.dma_start(out=ancf_t, in_=ancf)

            for b in range(BPC):
                in0 = inpool.tile([128, G2], F32, tag="in0")
                in1 = inpool.tile([127, G2], F32, tag="in1")
                # HWDGE HBM->SBUF loads assign descriptor i to SDMA slot
                # (i mod 32), and slots 16..31 all alias onto engine 0
                # (measured: a 128-desc load puts exactly 68/128 descriptors
                # on that engine -> 27 GB/s).  Capping each op at 16
                # descriptors (16 partition rows x one 11.5 KB descriptor)
                # keeps every op on slots 0..15 = all 16 engines in parallel.
                for lo, hi in ((0, G2 // 2), (G2 // 2, G2)):
                    for p0 in range(0, 128, 16):
                        nc.sync.dma_start(
                            out=in0[p0 : p0 + 16, lo:hi],
                            in_=pred[b, p0 : p0 + 16, lo:hi],
                        )
                    for p0 in range(0, 127, 16):
                        p1 = min(p0 + 16, 127)
                        nc.sync.dma_start(
                            out=in1[p0:p1, lo:hi],
                            in_=pred[b, 128 + p0 : 128 + p1, lo:hi],
                        )

                for j0, nb in GROUPS:
                    ps = pspool.tile([128, 8 * PSTRIDE], F32, tag="ps")
                    outt = outpool.tile([128, 8 * 255], F32, tag="outt")
                    wht = whpool.tile([128, 8 * 6], F32, tag="wht")
                    for jj in range(nb):
                        g0 = (j0 + jj) * 128
                        gcnt = min(128, G2 - g0)
                        po = jj * PSTRIDE
                        if gcnt < 128:
                            # Tail block: the ACT/DVE ops below read all 128
                            # partitions of this column range; zero it first
                            # (engine ops can't start at partition 16) so
                            # nothing reads stale PSUM, then let the
                            # transposes overwrite rows 0..gcnt.  Rows >=
                            # gcnt are never stored to DRAM.
                            nc.vector.memset(ps[:, po : po + 255], 0.0)
                        nc.tensor.transpose(
                            ps[0:gcnt, po : po + 128],
                            in0[:, g0 : g0 + gcnt],
                            ident_t,
                        )
                        nc.tensor.transpose(
                            ps[0:gcnt, po + 128 : po + 255],
                            in1[:, g0 : g0 + gcnt],
                            ident_t[0:127, 0:127],
                        )
                    nw = nb * 255
                    ps_v = ps[:, 0 : nb * PSTRIDE].rearrange(
                        "p (j c) -> p j c", c=PSTRIDE
                    )[:, :, 0:255]
                    out_v = outt[:, 0:nw].rearrange("p (j c) -> p j c", c=255)
                    nc.scalar.activation(
                        out_v, ps_v, mybir.ActivationFunctionType.Tanh, scale=0.5
                    )
                    ps_wh = ps_v.rearrange("p j (a k) -> p j a k", a=A)[:, :, :, 2:4]
                    wh_v = wht[:, 0 : nb * 6].rearrange(
                        "p (j a k) -> p j a k", a=A, k=2
                    )
                    nc.scalar.activation(
                        wh_v, ps_wh, mybir.ActivationFunctionType.Exp
                    )
                    # sigmoid = 0.5*tanh + 0.5, fused single pass
                    nc.vector.tensor_scalar(
                        out=outt[:, 0:nw],
                        in0=outt[:, 0:nw],
                        scalar1=0.5,
                        scalar2=0.5,
                        op0=mybir.AluOpType.mult,
                        op1=mybir.AluOpType.add,
                    )
                    out4 = out_v.rearrange("p j (a k) -> p j a k", a=A)
                    xy = out4[:, :, :, 0:2]
                    nc.vector.tensor_scalar_mul(xy, xy, float(stride))
                    gxy_v = gxy_t[:, j0 * 6 : (j0 + nb) * 6].rearrange(
                        "p (j a k) -> p j a k", a=A, k=2
                    )
                    nc.vector.tensor_add(xy, xy, gxy_v)
                    whc = out4[:, :, :, 2:4]
                    anc_v = ancf_t[:, 0 : nb * 6].rearrange(
                        "p (j a k) -> p j a k", a=A, k=2
                    )
                    nc.vector.tensor_mul(whc, wh_v, anc_v)

                    nfull = nb if (j0 + nb) * 128 <= G2 else nb - 1
                    base = j0 * 128 * 255
                    dst = out_flat[b, base : base + nfull * 128 * 255].rearrange(
                        "(j p c) -> p j c", p=128, c=255
                    )
                    src = outt[:, 0 : nfull * 255].rearrange("p (j c) -> p j c", c=255)
                    nc.scalar.dma_start(out=dst, in_=src)
                    if nfull != nb:
                        tb = base + nfull * 128 * 255
                        dst_t = out_flat[b, tb : tb + TAIL * 255].rearrange(
                            "(p c) -> p c", c=255
                        )
                        nc.scalar.dma_start(
                            out=dst_t,
                            in_=outt[0:TAIL, nfull * 255 : (nfull + 1) * 255],
                        )
    nc.compile()
    return nc


def _tables(stride: float, anchors: np.ndarray):
    g = np.arange(46 * 128, dtype=np.int64)
    gx = (g % G).astype(np.float32) * stride
    gy = (g // G).astype(np.float32) * stride
    gx[g >= G2] = 0.0
    gy[g >= G2] = 0.0
    gxy = np.stack([gx.reshape(46, 128).T, gy.reshape(46, 128).T], axis=-1)
    gxy = np.repeat(gxy[:, :, None, :], A, axis=2)  # [128, 46, 3, 2]
    gxy = np.ascontiguousarray(gxy.reshape(128, 46 * 6), dtype=np.float32)
    ancf = np.ascontiguousarray(
        np.broadcast_to(
            anchors.astype(np.float32)[None, None], (128, 8, A, 2)
        ).reshape(128, 48)
    )
    ident = np.eye(128, dtype=np.float32)
    return gxy, ancf, ident


def get_program(stride: float):
    key = float(stride)
    if key not in _PROGRAMS:
        _PROGRAMS[key] = _build_program(key)
    return _PROGRAMS[key]


def core_inputs(prediction, anchors, inputDim):
    """Host-side prep: per-core input dicts (exposed for testing)."""
    pred = np.asarray(prediction, dtype=np.float32)
    anc = np.asarray(anchors, dtype=np.float32)
    input_dim = int(np.asarray(inputDim))
    assert pred.shape == (B, CH, G, G), pred.shape
    assert anc.shape == (A, 2), anc.shape
    stride = input_dim // G
    predf = pred.reshape(B, CH, G2)
    gxy, ancf, ident = _tables(float(stride), anc)
    in_maps = [
        {
            "pred": np.ascontiguousarray(predf[i * BPC : (i + 1) * BPC]),
            "gxy": gxy,
            "ancf": ancf,
            "ident": ident,
        }
        for i in range(NCORES)
    ]
    return in_maps, stride


def kernel(prediction, anchors, inputDim, numClasses):
    global LAST_RESULTS
    assert int(np.asarray(numClasses)) == ATT - 5
    in_maps, stride = core_inputs(prediction, anchors, inputDim)
    nc = get_program(float(stride))
    kwargs = {}
    if int(os.environ.get("KERNEL_TRACE", "0")):
        kwargs = dict(trace=True, trace_cores=[0])
    res = bass_utils.run_bass_kernel_spmd(
        nc, in_maps, core_ids=list(range(NCORES)), **kwargs
    )
    LAST_RESULTS = res
    return np.concatenate([r["out"] for r in res.results], axis=0)


# revision 9
# speedup vs baseline: 2.3306x; 1.1505x over previous
"""YOLO anchor-box decode (predictTransform) as a Trainium2 Bass/Tile kernel.

Input : prediction [32, 255, 76, 76] f32, anchors [3,2] f32, inputDim, numClasses
Output: [32, 17328, 85] f32   (decoded boxes in input-image pixel units)

Math per batch (stride = inputDim // 76 = 8, attrs = 85, A = 3 anchors):
  view [255, 5776] -> transpose -> [5776, 255] rows g = (gy*76+gx), cols (a, k)
  k=0: (sigmoid(x) + gx) * stride      k=1: (sigmoid(y) + gy) * stride
  k=2: exp(w) * anchors[a,0]           k=3: exp(h) * anchors[a,1]
  k>=4: sigmoid(.)

Distribution: pure data parallel over batch, 4 batches per core on 8 cores.

Per-core dataflow (memory-bound problem, ~47 MB traffic/core):
  1. DMA each batch's [255, 5776] into SBUF as [128, 5776] + [127, 5776].
  2. TensorE transposes 128x128 fp32 blocks (identity matmul) into PSUM,
     8 g-blocks per PSUM group tile (256-col stride keeps each matmul
     output inside one PSUM bank).
  3. ScalarE reads PSUM and writes SBUF output tiles applying tanh(x/2)
     (whole tile) and exp (w/h cols -> small staging tile).  Using tanh
     instead of sigmoid keeps every ACT op inside the single
     `exp_and_others` table set: sigmoid and exp live in different ACT
     table sets and each switch costs ~2.7us.
  4. VectorE: sigmoid = 0.5*tanh + 0.5 (one fused mul-add pass), then the
     x/y affine (x stride, + stride*grid offset from a precomputed table)
     and w/h anchor multiply.
  5. DMA out [128 g-rows, nb*255] tiles; HBM side is contiguous per row.
"""

import os

import numpy as np

import concourse.bacc as bacc
import concourse.bass_utils as bass_utils
import concourse.mybir as mybir
import concourse.tile as tile

F32 = mybir.dt.float32

B, CH, G, G2, A, ATT = 32, 255, 76, 5776, 3, 85
NCORES, BPC = 8, 4            # cores, batches per core
TAIL = G2 - 45 * 128          # 16 leftover grid cells per batch
GROUPS = [(j, 4) for j in range(0, 44, 4)] + [(44, 2)]  # (first block, nblocks)
PSTRIDE = 256                 # per-block PSUM column stride (bank-safe for 255 cols)

_PROGRAMS = {}
LAST_RESULTS = None


def _build_program(stride: float):
    nc = bacc.Bacc(
        "TRN2",
        target_bir_lowering=False,
        debug=False,
        enable_asserts=False,
        num_devices=NCORES,
    )
    pred = nc.dram_tensor("pred", [BPC, CH, G2], F32, kind="ExternalInput").ap()
    gxy = nc.dram_tensor("gxy", [128, 46 * 6], F32, kind="ExternalInput").ap()
    ancf = nc.dram_tensor("ancf", [128, 24], F32, kind="ExternalInput").ap()
    ident = nc.dram_tensor("ident", [128, 128], F32, kind="ExternalInput").ap()
    out = nc.dram_tensor("out", [BPC, G2 * A, ATT], F32, kind="ExternalOutput").ap()
    out_flat = out.rearrange("b r k -> b (r k)")

    with tile.TileContext(nc) as tc:
        with (
            tc.tile_pool(name="consts", bufs=1) as consts,
            tc.tile_pool(name="inpool", bufs=3) as inpool,
            tc.tile_pool(name="outpool", bufs=4) as outpool,
            tc.tile_pool(name="whpool", bufs=4) as whpool,
            tc.tile_pool(name="pspool", bufs=4, space="PSUM") as pspool,
        ):
            ident_t = consts.tile([128, 128], F32)
            nc.sync.dma_start(out=ident_t, in_=ident)
            gxy_t = consts.tile([128, 46 * 6], F32)
            nc.sync.dma_start(out=gxy_t, in_=gxy)
            ancf_t = consts.tile([128, 24], F32)
            nc.sync.dma_start(out=ancf_t, in_=ancf)

            for b in range(BPC):
                in0 = inpool.tile([128, G2], F32, tag="in0")
                in1 = inpool.tile([127, G2], F32, tag="in1")
                # HWDGE HBM->SBUF loads assign descriptor i to SDMA slot
                # (i mod 32), and slots 16..31 all alias onto engine 0
                # (measured: a 128-desc load puts exactly 68/128 descriptors
                # on that engine -> 27 GB/s).  Capping each op at 16
                # descriptors (16 partition rows x one 11.5 KB descriptor)
                # keeps every op on slots 0..15 = all 16 engines in parallel.
                for lo, hi in ((0, G2 // 2), (G2 // 2, G2)):
                    for p0 in range(0, 128, 16):
                        nc.sync.dma_start(
                            out=in0[p0 : p0 + 16, lo:hi],
                            in_=pred[b, p0 : p0 + 16, lo:hi],
                        )
                    for p0 in range(0, 127, 16):
                        p1 = min(p0 + 16, 127)
                        nc.sync.dma_start(
                            out=in1[p0:p1, lo:hi],
                            in_=pred[b, 128 + p0 : 128 + p1, lo:hi],
                        )

                for j0, nb in GROUPS:
                    ps = pspool.tile([128, 4 * PSTRIDE], F32, tag="ps")
                    outt = outpool.tile([128, 4 * 255], F32, tag="outt")
                    wht = whpool.tile([128, 4 * 6], F32, tag="wht")
                    for jj in range(nb):
                        g0 = (j0 + jj) * 128
                        gcnt = min(128, G2 - g0)
                        po = jj * PSTRIDE
                        if gcnt < 128:
                            # Tail block: the ACT/DVE ops below read all 128
                            # partitions of this column range; zero it first
                            # (engine ops can't start at partition 16) so
                            # nothing reads stale PSUM, then let the
                            # transposes overwrite rows 0..gcnt.  Rows >=
                            # gcnt are never stored to DRAM.
                            nc.vector.memset(ps[:, po : po + 255], 0.0)
                        nc.tensor.transpose(
                            ps[0:gcnt, po : po + 128],
                            in0[:, g0 : g0 + gcnt],
                            ident_t,
                        )
                        nc.tensor.transpose(
                            ps[0:gcnt, po + 128 : po + 255],
                            in1[:, g0 : g0 + gcnt],
                            ident_t[0:127, 0:127],
                        )
                    nw = nb * 255
                    ps_v = ps[:, 0 : nb * PSTRIDE].rearrange(
                        "p (j c) -> p j c", c=PSTRIDE
                    )[:, :, 0:255]
                    out_v = outt[:, 0:nw].rearrange("p (j c) -> p j c", c=255)
                    nc.scalar.activation(
                        out_v, ps_v, mybir.ActivationFunctionType.Tanh, scale=0.5
                    )
                    ps_wh = ps_v.rearrange("p j (a k) -> p j a k", a=A)[:, :, :, 2:4]
                    wh_v = wht[:, 0 : nb * 6].rearrange(
                        "p (j a k) -> p j a k", a=A, k=2
                    )
                    nc.scalar.activation(
                        wh_v, ps_wh, mybir.ActivationFunctionType.Exp
                    )
                    # sigmoid = 0.5*tanh + 0.5, fused single pass
                    nc.vector.tensor_scalar(
                        out=outt[:, 0:nw],
                        in0=outt[:, 0:nw],
                        scalar1=0.5,
                        scalar2=0.5,
                        op0=mybir.AluOpType.mult,
                        op1=mybir.AluOpType.add,
                    )
                    out4 = out_v.rearrange("p j (a k) -> p j a k", a=A)
                    xy = out4[:, :, :, 0:2]
                    nc.vector.tensor_scalar_mul(xy, xy, float(stride))
                    gxy_v = gxy_t[:, j0 * 6 : (j0 + nb) * 6].rearrange(
                        "p (j a k) -> p j a k", a=A, k=2
                    )
                    nc.vector.tensor_add(xy, xy, gxy_v)
                    whc = out4[:, :, :, 2:4]
                    anc_v = ancf_t[:, 0 : nb * 6].rearrange(
                        "p (j a k) -> p j a k", a=A, k=2
                    )
                    nc.vector.tensor_mul(whc, wh_v, anc_v)

                    nfull = nb if (j0 + nb) * 128 <= G2 else nb - 1
                    base = j0 * 128 * 255
                    dst = out_flat[b, base : base + nfull * 128 * 255].rearrange(
                        "(j p c) -> p j c", p=128, c=255
                    )
                    src = outt[:, 0 : nfull * 255].rearrange("p (j c) -> p j c", c=255)
                    nc.scalar.dma_start(out=dst, in_=src)
                    if nfull != nb:
                        tb = base + nfull * 128 * 255
                        dst_t = out_flat[b, tb : tb + TAIL * 255].rearrange(
                            "(p c) -> p c", c=255
                        )
                        nc.scalar.dma_start(
                            out=dst_t,
                            in_=outt[0:TAIL, nfull * 255 : (nfull + 1) * 255],
                        )
    nc.compile()
    return nc


def _tables(stride: float, anchors: np.ndarray):
    g = np.arange(46 * 128, dtype=np.int64)
    gx = (g % G).astype(np.float32) * stride
    gy = (g // G).astype(np.float32) * stride
    gx[g >= G2] = 0.0
    gy[g >= G2] = 0.0
    gxy = np.stack([gx.reshape(46, 128).T, gy.reshape(46, 128).T], axis=-1)
    gxy = np.repeat(gxy[:, :, None, :], A, axis=2)  # [128, 46, 3, 2]
    gxy = np.ascontiguousarray(gxy.reshape(128, 46 * 6), dtype=np.float32)
    ancf = np.ascontiguousarray(
        np.broadcast_to(
            anchors.astype(np.float32)[None, None], (128, 4, A, 2)
        ).reshape(128, 24)
    )
    ident = np.eye(128, dtype=np.float32)
    return gxy, ancf, ident


def get_program(stride: float):
    key = float(stride)
    if key not in _PROGRAMS:
        _PROGRAMS[key] = _build_program(key)
    return _PROGRAMS[key]


def core_inputs(prediction, anchors, inputDim):
    """Host-side prep: per-core input dicts (exposed for testing)."""
    pred = np.asarray(prediction, dtype=np.float32)
    anc = np.asarray(anchors, dtype=np.float32)
    input_dim = int(np.asarray(inputDim))
    assert pred.shape == (B, CH, G, G), pred.shape
    assert anc.shape == (A, 2), anc.shape
    stride = input_dim // G
    predf = pred.reshape(B, CH, G2)
    gxy, ancf, ident = _tables(float(stride), anc)
    in_maps = [
        {
            "pred": np.ascontiguousarray(predf[i * BPC : (i + 1) * BPC]),
            "gxy": gxy,
            "ancf": ancf,
            "ident": ident,
        }
        for i in range(NCORES)
    ]
    return in_maps, stride


def kernel(prediction, anchors, inputDim, numClasses):
    global LAST_RESULTS
    assert int(np.asarray(numClasses)) == ATT - 5
    in_maps, stride = core_inputs(prediction, anchors, inputDim)
    nc = get_program(float(stride))
    kwargs = {}
    if int(os.environ.get("KERNEL_TRACE", "0")):
        kwargs = dict(trace=True, trace_cores=[0])
    res = bass_utils.run_bass_kernel_spmd(
        nc, in_maps, core_ids=list(range(NCORES)), **kwargs
    )
    LAST_RESULTS = res
    return np.concatenate([r["out"] for r in res.results], axis=0)
